# revision 8
# baseline (speedup 1.0000x reference)
"""Cross-attention kernel for Trainium2 (8 NeuronCores, SPMD).

Problem: q [2, 2048, 16, 64], kv [2, 2048, 2, 16, 64] (k=kv[:,:,0], v=kv[:,:,1])
  scores = einsum('bthd,bshd->bhts', q, k/sqrt(d)); P = softmax(scores, -1)
  out = einsum('bhts,bshd->bthd', P, v)    -> [2, 2048, 16, 64]

Sharding: 32 (b,h) heads across 8 cores -> 4 heads/core (data parallel on b,
tensor parallel on h; no communication).

Per-core algorithm (per head, t=s=2048, d=64), bf16 matmul datapath:
  - Host pre-lays-out one combined bf16 tensor per head: Q^T [64,2048]
    duplicated into both PE row halves, K^T*scale packed so even s-tiles sit
    at partitions 0-63 and odd s-tiles at 64-127 (2-way row-packed matmuls),
    and V' = [V, 1] (ones column yields the softmax denominator).
  - S^T supertile [128, 1024] fp32 PSUM (2 banks) holds the j-pair: s-tile 2j
    at cols 0:512, s-tile 2j+1 at cols 512:1024, same t-quarter. The two QK
    matmuls issue adjacently into PE row groups 0-63 / 64-127 (concurrent).
  - P^T = exp(S^T): supertiles are split between ScalarE (native Exp ACTIVATE,
    FD=1024) and VectorE (Schraudolph bit-trick: int16(x*A+B) bits == bf16
    exp2(x/ln2), +-3% relative) so exp throughput ~ sums the two engines.
    No max subtraction: scores are N(0,1)-ish, |s| < ~8, exp fits bf16, and
    numerator/denominator share the same approximation so softmax stays
    consistent.
  - O'^T [65, t] += V'_i^T @ P^T_i accumulated over s-tiles in PSUM.
    Rows 0-63 = unnormalized O^T, row 64 = sum_s exp = softmax denominator.
  - PE-transpose 128-col chunks of O'^T -> [128, 65]; out = cols 0-63 times
    reciprocal(col 64) on VectorE; DMA to DRAM in [t, h, d] layout.
"""

import math

import numpy as np

import concourse.bass as bass
from concourse import bacc
import concourse.mybir as mybir
import concourse.tile as tile
from concourse.bass_utils import run_bass_kernel_spmd

B, T, H, D = 2, 2048, 16, 64
N_CORES = 8
HPC = (B * H) // N_CORES  # heads per core = 4
P = 128
NS = T // P  # 16 s-tiles
SCALE = 1.0 / math.sqrt(D)
F32 = mybir.dt.float32
BF16 = mybir.dt.bfloat16
I16 = mybir.dt.int16
NP_BF16 = mybir.dt.np(BF16)

# Schraudolph exp in bf16-bits domain: int16(round(x*EXPA + EXPB)) viewed as
# bf16 ~= exp(x), max rel err ~3% (c=0.0431 centers the sawtooth).
EXPA = 128.0 / math.log(2.0)
EXPB = 16256.0 - 128.0 * 0.0431

# Which j-pair supertiles (of 8 per (head, t-quarter)) go to VectorE instead
# of ScalarE for the exp.
DVE_JS = (1, 4, 6)

# Combined per-head input layout (per partition): [ Q^T 2048 | K^T 1024 | V' 1040 ]
KT_OFF = T
VP_OFF = T + (NS // 2) * P
INP_W = VP_OFF + NS * (D + 1)

LAST_RESULT = None  # BassKernelResults of the most recent kernel() call
_BASS_CACHE = {}


def _build_bass():
    nc = bacc.Bacc("TRN2", target_bir_lowering=False)

    inp_d = nc.declare_dram_parameter("inp", [HPC, P, INP_W], BF16, isOutput=False)
    out_d = nc.declare_dram_parameter("out", [T, HPC, D], F32, isOutput=True)

    ident_d = nc.inline_tensor(np.eye(P, dtype=np.float32), name="ident")

    TW = 512  # t-quarter per inner pass (PSUM bank width in fp32)

    with tile.TileContext(nc) as tc:
        with (
            tc.tile_pool(name="const", bufs=1) as cpool,
            tc.tile_pool(name="heads", bufs=2) as hpool,
            tc.tile_pool(name="pt", bufs=12) as ptpool,
            tc.tile_pool(name="outs", bufs=2) as opool,
            tc.tile_pool(name="spsum", bufs=3, space="PSUM") as spsum,
            tc.tile_pool(name="opsum", bufs=1, space="PSUM") as opsum,
            tc.tile_pool(name="tpsum", bufs=1, space="PSUM") as tpsum,
        ):
            id_sb = cpool.tile([P, P], F32)
            nc.sync.dma_start(id_sb[:], ident_d.ap())
            # Dummy transpose: absorbs the ident-DMA wait on the PE engine so
            # later (wait-limited) matmul/transpose instructions never need it.
            tp0 = tpsum.tile([P, 4, D + 4], F32, tag="tp")
            nc.tensor.transpose(
                tp0[:, 0, : D + 1], id_sb[: D + 1, :], id_sb[: D + 1, : D + 1]
            )

            # PE warm-up: dummy matmuls (~5us) issued while the first input
            # DMA is in flight, so the clock-gate reaches K=8/8 before the
            # real matmul stream starts.
            wu = cpool.tile([P, 640], BF16)
            nc.gpsimd.memset(wu[:], 0.0)
            for _w in range(16):
                wups = spsum.tile([P, 2 * TW], F32, tag="ps")
                nc.tensor.matmul(
                    wups[:, 0:TW],
                    lhsT=wu[0:64, 0:P],
                    rhs=wu[0:64, P : P + TW],
                    start=True,
                    stop=True,
                )

            out_view = out_d.ap().rearrange("(c p) hh d -> p c hh d", p=P)

            for hh in range(HPC):
                inp_sb = hpool.tile([P, INP_W], BF16, tag="inp")
                nc.sync.dma_start(inp_sb[:], inp_d.ap()[hh])
                qt_sb = inp_sb[:, 0:T]

                def kt_sb(j):  # K^T chunk j: [128, 128]
                    return inp_sb[:, KT_OFF + j * P : KT_OFF + (j + 1) * P]

                def vp_sb(i):  # V' s-tile i: [128, 65]
                    return inp_sb[:, VP_OFF + i * (D + 1) : VP_OFF + (i + 1) * (D + 1)]

                for th in range(T // TW):
                    ps_o = opsum.tile([D + 1, TW], F32, tag="po")
                    tsl = slice(th * TW, (th + 1) * TW)

                    # Software-pipelined j loop: QK+exp run LOOKAHEAD pairs
                    # ahead of PV in program order, so a slow exp never
                    # head-of-line-blocks the PE queue (which would starve
                    # ScalarE of fresh score tiles).
                    LOOKAHEAD = 2
                    pts = {}

                    def emit_qk_exp(j):
                        st = spsum.tile([P, 2 * TW], F32, tag="ps")
                        # S^T = K_tile @ Q^T; adjacent h0/h64 issue -> the two
                        # s-tiles run concurrently in PE row groups.
                        nc.tensor.matmul(
                            st[:, 0:TW],
                            lhsT=kt_sb(j)[0:64, :],
                            rhs=qt_sb[0:64, tsl],
                            start=True,
                            stop=True,
                        )
                        nc.tensor.matmul(
                            st[:, TW : 2 * TW],
                            lhsT=kt_sb(j)[64:128, :],
                            rhs=qt_sb[64:128, tsl],
                            start=True,
                            stop=True,
                        )
                        pt = ptpool.tile([P, 2 * TW], BF16, tag="pt")
                        if j in DVE_JS:
                            nc.vector.tensor_scalar(
                                pt[:].bitcast(I16),
                                st[:],
                                EXPA,
                                EXPB,
                                mybir.AluOpType.mult,
                                mybir.AluOpType.add,
                            )
                        else:
                            nc.scalar.activation(
                                pt[:], st[:], mybir.ActivationFunctionType.Exp
                            )
                        pts[j] = pt

                    def emit_pv(j):
                        pt = pts.pop(j)
                        nc.tensor.matmul(
                            ps_o[:],
                            lhsT=vp_sb(2 * j),
                            rhs=pt[:, 0:TW],
                            start=(j == 0),
                            stop=False,
                        )
                        nc.tensor.matmul(
                            ps_o[:],
                            lhsT=vp_sb(2 * j + 1),
                            rhs=pt[:, TW : 2 * TW],
                            start=False,
                            stop=(j == NS // 2 - 1),
                        )

                    for j in range(NS // 2):  # s-tile pairs (2j, 2j+1)
                        emit_qk_exp(j)
                        if j >= LOOKAHEAD:
                            emit_pv(j - LOOKAHEAD)
                    for j in range(NS // 2 - LOOKAHEAD, NS // 2):
                        emit_pv(j)

                    # Normalize + emit this (head, t-quarter). All 4 transposes
                    # land in one padded 1-bank PSUM tile so they run
                    # back-to-back on the PE with a single DVE handoff after.
                    o_sb = opool.tile([D + 1, TW], F32, tag="osb")
                    nc.vector.tensor_copy(o_sb[:], ps_o[:])
                    ostage = opool.tile([P, TW // P, D], F32, tag="ost")
                    rec = opool.tile([P, TW // P], F32, tag="rec")
                    tp4 = tpsum.tile([P, TW // P, D + 4], F32, tag="tp")
                    for cc in range(TW // P):
                        nc.tensor.transpose(
                            tp4[:, cc, : D + 1],
                            o_sb[:, cc * P : (cc + 1) * P],
                            id_sb[: D + 1, : D + 1],
                        )
                    nc.vector.reciprocal(rec[:], tp4[:, :, D])
                    nc.vector.tensor_mul(
                        ostage[:],
                        tp4[:, :, 0:D],
                        rec[:, :, None].broadcast_to([P, TW // P, D]),
                    )
                    nc.sync.dma_start(
                        out_view[:, th * (TW // P) : (th + 1) * (TW // P), hh, :],
                        ostage[:],
                    )

    nc.compile()
    return nc


def get_bass():
    if "nc" not in _BASS_CACHE:
        _BASS_CACHE["nc"] = _build_bass()
    return _BASS_CACHE["nc"]


def make_core_inputs(q, kv, core):
    """Host-side sharding + layout for one core: returns {inp}."""
    b = core // (N_CORES // B)
    h0 = HPC * (core % (N_CORES // B))
    inp = np.empty((HPC, P, INP_W), NP_BF16)
    for i in range(HPC):
        h = h0 + i
        Qt = np.ascontiguousarray(q[b, :, h, :].T)  # [64, 2048]
        inp[i, :64, 0:T] = Qt
        inp[i, 64:, 0:T] = Qt
        Kt = (kv[b, :, 0, h, :].astype(np.float32) * SCALE).T  # [64, 2048]
        Kts = Kt.reshape(64, NS, P)
        kt = inp[i, :, KT_OFF:VP_OFF].reshape(P, NS // 2, P)
        kt[:64] = Kts[:, 0::2]  # even s-tiles -> partitions 0-63
        kt[64:] = Kts[:, 1::2]  # odd s-tiles -> partitions 64-127
        V = kv[b, :, 1, h, :].reshape(NS, P, D)  # [s_tile, p, d]
        vp = inp[i, :, VP_OFF:].reshape(P, NS, D + 1)
        vp[:, :, :D] = V.transpose(1, 0, 2)
        vp[:, :, D] = 1.0
    return {"inp": inp}


def kernel(q, kv):
    global LAST_RESULT
    q = np.asarray(q, dtype=np.float32)
    kv = np.asarray(kv, dtype=np.float32)
    assert q.shape == (B, T, H, D) and kv.shape == (B, T, 2, H, D)

    nc = get_bass()
    in_maps = [make_core_inputs(q, kv, c) for c in range(N_CORES)]
    res = run_bass_kernel_spmd(nc, in_maps, core_ids=list(range(N_CORES)))
    LAST_RESULT = res

    out = np.empty((B, T, H, D), np.float32)
    for c in range(N_CORES):
        b = c // (N_CORES // B)
        h0 = HPC * (c % (N_CORES // B))
        out[b, :, h0 : h0 + HPC, :] = res.results[c]["out"]
    return out


# revision 9
# speedup vs baseline: 1.0964x; 1.0964x over previous
"""Cross-attention kernel for Trainium2 (8 NeuronCores, SPMD).

Problem: q [2, 2048, 16, 64], kv [2, 2048, 2, 16, 64] (k=kv[:,:,0], v=kv[:,:,1])
  scores = einsum('bthd,bshd->bhts', q, k/sqrt(d)); P = softmax(scores, -1)
  out = einsum('bhts,bshd->bthd', P, v)    -> [2, 2048, 16, 64]

Sharding: 32 (b,h) heads across 8 cores -> 4 heads/core (data parallel on b,
tensor parallel on h; no communication).

Per-core algorithm (per head, t=s=2048, d=64), bf16 matmul datapath:
  - Host pre-lays-out one combined bf16 tensor per head: Q^T [64,2048]
    duplicated into both PE row halves, K^T*scale packed so even s-tiles sit
    at partitions 0-63 and odd s-tiles at 64-127 (2-way row-packed matmuls),
    and V' = [V, 1] (ones column yields the softmax denominator).
  - S^T supertile [128, 1024] fp32 PSUM (2 banks) holds the j-pair: s-tile 2j
    at cols 0:512, s-tile 2j+1 at cols 512:1024, same t-quarter. The two QK
    matmuls issue adjacently into PE row groups 0-63 / 64-127 (concurrent).
  - P^T = exp(S^T): supertiles are split between ScalarE (native Exp ACTIVATE,
    FD=1024) and VectorE (Schraudolph bit-trick: int16(x*A+B) bits == bf16
    exp2(x/ln2), +-3% relative) so exp throughput ~ sums the two engines.
    No max subtraction: scores are N(0,1)-ish, |s| < ~8, exp fits bf16, and
    numerator/denominator share the same approximation so softmax stays
    consistent.
  - O'^T [65, t] += V'_i^T @ P^T_i accumulated over s-tiles in PSUM.
    Rows 0-63 = unnormalized O^T, row 64 = sum_s exp = softmax denominator.
  - PE-transpose 128-col chunks of O'^T -> [128, 65]; out = cols 0-63 times
    reciprocal(col 64) on VectorE; DMA to DRAM in [t, h, d] layout.
"""

import math

import numpy as np

import concourse.bass as bass
from concourse import bacc
import concourse.mybir as mybir
import concourse.tile as tile
from concourse.bass_utils import run_bass_kernel_spmd

B, T, H, D = 2, 2048, 16, 64
N_CORES = 8
HPC = (B * H) // N_CORES  # heads per core = 4
P = 128
NS = T // P  # 16 s-tiles
SCALE = 1.0 / math.sqrt(D)
F32 = mybir.dt.float32
BF16 = mybir.dt.bfloat16
I16 = mybir.dt.int16
NP_BF16 = mybir.dt.np(BF16)

# Schraudolph exp in bf16-bits domain: int16(round(x*EXPA + EXPB)) viewed as
# bf16 ~= exp(x), max rel err ~3% (c=0.0431 centers the sawtooth).
EXPA = 128.0 / math.log(2.0)
EXPB = 16256.0 - 128.0 * 0.0431

# Which j-pair supertiles (of 8 per (head, t-quarter)) go to VectorE instead
# of ScalarE for the exp.
DVE_JS = (2, 5)
DVE_HALF_J = 3

# Combined per-head input layout (per partition): [ Q^T 2048 | K^T 1024 | V' 1040 ]
KT_OFF = T
VP_OFF = T + (NS // 2) * P
INP_W = VP_OFF + NS * (D + 1)

LAST_RESULT = None  # BassKernelResults of the most recent kernel() call
_BASS_CACHE = {}


def _build_bass():
    nc = bacc.Bacc("TRN2", target_bir_lowering=False)

    inp_d = nc.declare_dram_parameter("inp", [HPC, P, INP_W], BF16, isOutput=False)
    out_d = nc.declare_dram_parameter("out", [T, HPC, D], F32, isOutput=True)

    ident_d = nc.inline_tensor(np.eye(P, dtype=np.float32), name="ident")

    TW = 512  # t-quarter per inner pass (PSUM bank width in fp32)

    with tile.TileContext(nc) as tc:
        with (
            tc.tile_pool(name="const", bufs=1) as cpool,
            tc.tile_pool(name="heads", bufs=2) as hpool,
            tc.tile_pool(name="pt", bufs=12) as ptpool,
            tc.tile_pool(name="outs", bufs=2) as opool,
            tc.tile_pool(name="spsum", bufs=3, space="PSUM") as spsum,
            tc.tile_pool(name="opsum", bufs=1, space="PSUM") as opsum,
            tc.tile_pool(name="tpsum", bufs=1, space="PSUM") as tpsum,
        ):
            id_sb = cpool.tile([P, P], F32)
            nc.sync.dma_start(id_sb[:], ident_d.ap())
            # Dummy transpose: absorbs the ident-DMA wait on the PE engine so
            # later (wait-limited) matmul/transpose instructions never need it.
            tp0 = tpsum.tile([P, 4, D + 4], F32, tag="tp")
            nc.tensor.transpose(
                tp0[:, 0, : D + 1], id_sb[: D + 1, :], id_sb[: D + 1, : D + 1]
            )

            # PE warm-up: dummy matmuls (~5us) issued while the first input
            # DMA is in flight, so the clock-gate reaches K=8/8 before the
            # real matmul stream starts.
            wu = cpool.tile([P, 640], BF16)
            nc.gpsimd.memset(wu[:], 0.0)
            for _w in range(8):
                wups = spsum.tile([P, 2 * TW], F32, tag="ps")
                nc.tensor.matmul(
                    wups[:, 0:TW],
                    lhsT=wu[0:64, 0:P],
                    rhs=wu[0:64, P : P + TW],
                    start=True,
                    stop=True,
                )

            out_view = out_d.ap().rearrange("(c p) hh d -> p c hh d", p=P)

            for hh in range(HPC):
                inp_sb = hpool.tile([P, INP_W], BF16, tag="inp")
                nc.sync.dma_start(inp_sb[:], inp_d.ap()[hh])
                qt_sb = inp_sb[:, 0:T]

                def kt_sb(j):  # K^T chunk j: [128, 128]
                    return inp_sb[:, KT_OFF + j * P : KT_OFF + (j + 1) * P]

                def vp_sb(i):  # V' s-tile i: [128, 65]
                    return inp_sb[:, VP_OFF + i * (D + 1) : VP_OFF + (i + 1) * (D + 1)]

                for th in range(T // TW):
                    ps_o = opsum.tile([D + 1, TW], F32, tag="po")
                    tsl = slice(th * TW, (th + 1) * TW)

                    # Software-pipelined j loop: QK+exp run LOOKAHEAD pairs
                    # ahead of PV in program order, so a slow exp never
                    # head-of-line-blocks the PE queue (which would starve
                    # ScalarE of fresh score tiles).
                    LOOKAHEAD = 2
                    pts = {}

                    def emit_qk_exp(j):
                        st = spsum.tile([P, 2 * TW], F32, tag="ps")
                        # S^T = K_tile @ Q^T; adjacent h0/h64 issue -> the two
                        # s-tiles run concurrently in PE row groups.
                        nc.tensor.matmul(
                            st[:, 0:TW],
                            lhsT=kt_sb(j)[0:64, :],
                            rhs=qt_sb[0:64, tsl],
                            start=True,
                            stop=True,
                        )
                        nc.tensor.matmul(
                            st[:, TW : 2 * TW],
                            lhsT=kt_sb(j)[64:128, :],
                            rhs=qt_sb[64:128, tsl],
                            start=True,
                            stop=True,
                        )
                        pt = ptpool.tile([P, 2 * TW], BF16, tag="pt")
                        if j in DVE_JS:
                            nc.vector.tensor_scalar(
                                pt[:].bitcast(I16),
                                st[:],
                                EXPA,
                                EXPB,
                                mybir.AluOpType.mult,
                                mybir.AluOpType.add,
                            )
                        elif j == DVE_HALF_J:
                            nc.scalar.activation(
                                pt[:, 0:TW], st[:, 0:TW],
                                mybir.ActivationFunctionType.Exp,
                            )
                            nc.vector.tensor_scalar(
                                pt[:, TW : 2 * TW].bitcast(I16),
                                st[:, TW : 2 * TW],
                                EXPA,
                                EXPB,
                                mybir.AluOpType.mult,
                                mybir.AluOpType.add,
                            )
                        else:
                            nc.scalar.activation(
                                pt[:], st[:], mybir.ActivationFunctionType.Exp
                            )
                        pts[j] = pt

                    def emit_pv(j):
                        pt = pts.pop(j)
                        nc.tensor.matmul(
                            ps_o[:],
                            lhsT=vp_sb(2 * j),
                            rhs=pt[:, 0:TW],
                            start=(j == 0),
                            stop=False,
                        )
                        nc.tensor.matmul(
                            ps_o[:],
                            lhsT=vp_sb(2 * j + 1),
                            rhs=pt[:, TW : 2 * TW],
                            start=False,
                            stop=(j == NS // 2 - 1),
                        )

                    for j in range(NS // 2):  # s-tile pairs (2j, 2j+1)
                        emit_qk_exp(j)
                        if j >= LOOKAHEAD:
                            emit_pv(j - LOOKAHEAD)
                    for j in range(NS // 2 - LOOKAHEAD, NS // 2):
                        emit_pv(j)

                    # Normalize + emit this (head, t-quarter). All 4 transposes
                    # land in one padded 1-bank PSUM tile so they run
                    # back-to-back on the PE with a single DVE handoff after.
                    o_sb = opool.tile([D + 1, TW], F32, tag="osb")
                    nc.vector.tensor_copy(o_sb[:], ps_o[:])
                    ostage = opool.tile([P, TW // P, D], F32, tag="ost")
                    rec = opool.tile([P, TW // P], F32, tag="rec")
                    tp4 = tpsum.tile([P, TW // P, D + 4], F32, tag="tp")
                    for cc in range(TW // P):
                        nc.tensor.transpose(
                            tp4[:, cc, : D + 1],
                            o_sb[:, cc * P : (cc + 1) * P],
                            id_sb[: D + 1, : D + 1],
                        )
                    nc.vector.reciprocal(rec[:], tp4[:, :, D])
                    nc.vector.tensor_mul(
                        ostage[:],
                        tp4[:, :, 0:D],
                        rec[:, :, None].broadcast_to([P, TW // P, D]),
                    )
                    nc.sync.dma_start(
                        out_view[:, th * (TW // P) : (th + 1) * (TW // P), hh, :],
                        ostage[:],
                    )

    nc.compile()
    return nc


def get_bass():
    if "nc" not in _BASS_CACHE:
        _BASS_CACHE["nc"] = _build_bass()
    return _BASS_CACHE["nc"]


def make_core_inputs(q, kv, core):
    """Host-side sharding + layout for one core: returns {inp}."""
    b = core // (N_CORES // B)
    h0 = HPC * (core % (N_CORES // B))
    inp = np.empty((HPC, P, INP_W), NP_BF16)
    for i in range(HPC):
        h = h0 + i
        Qt = np.ascontiguousarray(q[b, :, h, :].T)  # [64, 2048]
        inp[i, :64, 0:T] = Qt
        inp[i, 64:, 0:T] = Qt
        Kt = (kv[b, :, 0, h, :].astype(np.float32) * SCALE).T  # [64, 2048]
        Kts = Kt.reshape(64, NS, P)
        kt = inp[i, :, KT_OFF:VP_OFF].reshape(P, NS // 2, P)
        kt[:64] = Kts[:, 0::2]  # even s-tiles -> partitions 0-63
        kt[64:] = Kts[:, 1::2]  # odd s-tiles -> partitions 64-127
        V = kv[b, :, 1, h, :].reshape(NS, P, D)  # [s_tile, p, d]
        vp = inp[i, :, VP_OFF:].reshape(P, NS, D + 1)
        vp[:, :, :D] = V.transpose(1, 0, 2)
        vp[:, :, D] = 1.0
    return {"inp": inp}


def kernel(q, kv):
    global LAST_RESULT
    q = np.asarray(q, dtype=np.float32)
    kv = np.asarray(kv, dtype=np.float32)
    assert q.shape == (B, T, H, D) and kv.shape == (B, T, 2, H, D)

    nc = get_bass()
    in_maps = [make_core_inputs(q, kv, c) for c in range(N_CORES)]
    res = run_bass_kernel_spmd(nc, in_maps, core_ids=list(range(N_CORES)))
    LAST_RESULT = res

    out = np.empty((B, T, H, D), np.float32)
    for c in range(N_CORES):
        b = c // (N_CORES // B)
        h0 = HPC * (c % (N_CORES // B))
        out[b, :, h0 : h0 + HPC, :] = res.results[c]["out"]
    return out


# revision 10
# speedup vs baseline: 1.0984x; 1.0018x over previous
"""Cross-attention kernel for Trainium2 (8 NeuronCores, SPMD).

Problem: q [2, 2048, 16, 64], kv [2, 2048, 2, 16, 64] (k=kv[:,:,0], v=kv[:,:,1])
  scores = einsum('bthd,bshd->bhts', q, k/sqrt(d)); P = softmax(scores, -1)
  out = einsum('bhts,bshd->bthd', P, v)    -> [2, 2048, 16, 64]

Sharding: 32 (b,h) heads across 8 cores -> 4 heads/core (data parallel on b,
tensor parallel on h; no communication).

Per-core algorithm (per head, t=s=2048, d=64), bf16 matmul datapath:
  - Host pre-lays-out one combined bf16 tensor per head: Q^T [64,2048]
    duplicated into both PE row halves, K^T*scale packed so even s-tiles sit
    at partitions 0-63 and odd s-tiles at 64-127 (2-way row-packed matmuls),
    and V' = [V, 1] (ones column yields the softmax denominator).
  - S^T supertile [128, 1024] fp32 PSUM (2 banks) holds the j-pair: s-tile 2j
    at cols 0:512, s-tile 2j+1 at cols 512:1024, same t-quarter. The two QK
    matmuls issue adjacently into PE row groups 0-63 / 64-127 (concurrent).
  - P^T = exp(S^T): supertiles are split between ScalarE (native Exp ACTIVATE,
    FD=1024) and VectorE (Schraudolph bit-trick: int16(x*A+B) bits == bf16
    exp2(x/ln2), +-3% relative) so exp throughput ~ sums the two engines.
    No max subtraction: scores are N(0,1)-ish, |s| < ~8, exp fits bf16, and
    numerator/denominator share the same approximation so softmax stays
    consistent.
  - O'^T [65, t] += V'_i^T @ P^T_i accumulated over s-tiles in PSUM.
    Rows 0-63 = unnormalized O^T, row 64 = sum_s exp = softmax denominator.
  - PE-transpose 128-col chunks of O'^T -> [128, 65]; out = cols 0-63 times
    reciprocal(col 64) on VectorE; DMA to DRAM in [t, h, d] layout.
"""

import math

import numpy as np

import concourse.bass as bass
from concourse import bacc
import concourse.mybir as mybir
import concourse.tile as tile
from concourse.bass_utils import run_bass_kernel_spmd

B, T, H, D = 2, 2048, 16, 64
N_CORES = 8
HPC = (B * H) // N_CORES  # heads per core = 4
P = 128
NS = T // P  # 16 s-tiles
SCALE = 1.0 / math.sqrt(D)
F32 = mybir.dt.float32
BF16 = mybir.dt.bfloat16
I16 = mybir.dt.int16
NP_BF16 = mybir.dt.np(BF16)

# Schraudolph exp in bf16-bits domain: int16(round(x*EXPA + EXPB)) viewed as
# bf16 ~= exp(x), max rel err ~3% (c=0.0431 centers the sawtooth).
EXPA = 128.0 / math.log(2.0)
EXPB = 16256.0 - 128.0 * 0.0431

# Which j-pair supertiles (of 8 per (head, t-quarter)) go to VectorE instead
# of ScalarE for the exp.
DVE_JS = (2, 5)
DVE_HALF_J = 3

# Combined per-head input layout (per partition): [ Q^T 2048 | K^T 1024 | V' 1040 ]
KT_OFF = T
VP_OFF = T + (NS // 2) * P
INP_W = VP_OFF + NS * (D + 1)

LAST_RESULT = None  # BassKernelResults of the most recent kernel() call
_BASS_CACHE = {}


def _build_bass():
    nc = bacc.Bacc("TRN2", target_bir_lowering=False)

    inp_d = nc.declare_dram_parameter("inp", [HPC, P, INP_W], BF16, isOutput=False)
    out_d = nc.declare_dram_parameter("out", [T, HPC, D], F32, isOutput=True)

    ident_d = nc.inline_tensor(np.eye(P, dtype=np.float32), name="ident")

    TW = 512  # t-quarter per inner pass (PSUM bank width in fp32)

    with tile.TileContext(nc) as tc:
        with (
            tc.tile_pool(name="const", bufs=1) as cpool,
            tc.tile_pool(name="heads", bufs=2) as hpool,
            tc.tile_pool(name="pt", bufs=12) as ptpool,
            tc.tile_pool(name="outs", bufs=2) as opool,
            tc.tile_pool(name="spsum", bufs=3, space="PSUM") as spsum,
            tc.tile_pool(name="opsum", bufs=1, space="PSUM") as opsum,
            tc.tile_pool(name="tpsum", bufs=1, space="PSUM") as tpsum,
        ):
            id_sb = cpool.tile([P, P], F32)
            nc.sync.dma_start(id_sb[:], ident_d.ap())
            # Dummy transpose: absorbs the ident-DMA wait on the PE engine so
            # later (wait-limited) matmul/transpose instructions never need it.
            tp0 = tpsum.tile([P, 4, D + 4], F32, tag="tp")
            nc.tensor.transpose(
                tp0[:, 0, : D + 1], id_sb[: D + 1, :], id_sb[: D + 1, : D + 1]
            )

            # PE warm-up: dummy matmuls (~5us) issued while the first input
            # DMA is in flight, so the clock-gate reaches K=8/8 before the
            # real matmul stream starts.
            wu = cpool.tile([P, 640], BF16)
            nc.gpsimd.memset(wu[:], 0.0)
            for _w in range(8):
                wups = spsum.tile([P, 2 * TW], F32, tag="ps")
                nc.tensor.matmul(
                    wups[:, 0:TW],
                    lhsT=wu[0:64, 0:P],
                    rhs=wu[0:64, P : P + TW],
                    start=True,
                    stop=True,
                )

            out_view = out_d.ap().rearrange("(c p) hh d -> p c hh d", p=P)

            # Deferred finalize: the PSUM-evacuate/normalize/store chain for
            # (hh, th) is emitted inside the NEXT tile's j-loop, after a few
            # QK+exp pairs, so its long sem-waits never head-of-line-block
            # the DVE/PE queues at the period boundary.
            pending_fin = [None]

            def emit_finalize():
                fin = pending_fin[0]
                if fin is None:
                    return
                pending_fin[0] = None
                ps_o, f_hh, f_th = fin
                o_sb = opool.tile([D + 1, TW], F32, tag="osb")
                nc.vector.tensor_copy(o_sb[:], ps_o[:])
                ostage = opool.tile([P, TW // P, D], F32, tag="ost")
                rec = opool.tile([P, TW // P], F32, tag="rec")
                tp4 = tpsum.tile([P, TW // P, D + 4], F32, tag="tp")
                for cc in range(TW // P):
                    nc.tensor.transpose(
                        tp4[:, cc, : D + 1],
                        o_sb[:, cc * P : (cc + 1) * P],
                        id_sb[: D + 1, : D + 1],
                    )
                nc.vector.reciprocal(rec[:], tp4[:, :, D])
                nc.vector.tensor_mul(
                    ostage[:],
                    tp4[:, :, 0:D],
                    rec[:, :, None].broadcast_to([P, TW // P, D]),
                )
                nc.sync.dma_start(
                    out_view[:, f_th * (TW // P) : (f_th + 1) * (TW // P), f_hh, :],
                    ostage[:],
                )

            for hh in range(HPC):
                inp_sb = hpool.tile([P, INP_W], BF16, tag="inp")
                nc.sync.dma_start(inp_sb[:], inp_d.ap()[hh])
                qt_sb = inp_sb[:, 0:T]

                def kt_sb(j):  # K^T chunk j: [128, 128]
                    return inp_sb[:, KT_OFF + j * P : KT_OFF + (j + 1) * P]

                def vp_sb(i):  # V' s-tile i: [128, 65]
                    return inp_sb[:, VP_OFF + i * (D + 1) : VP_OFF + (i + 1) * (D + 1)]

                for th in range(T // TW):
                    ps_o = opsum.tile([D + 1, TW], F32, tag="po")
                    tsl = slice(th * TW, (th + 1) * TW)

                    # Software-pipelined j loop: QK+exp run LOOKAHEAD pairs
                    # ahead of PV in program order, so a slow exp never
                    # head-of-line-blocks the PE queue (which would starve
                    # ScalarE of fresh score tiles).
                    LOOKAHEAD = 2
                    pts = {}

                    def emit_qk_exp(j):
                        st = spsum.tile([P, 2 * TW], F32, tag="ps")
                        # S^T = K_tile @ Q^T; adjacent h0/h64 issue -> the two
                        # s-tiles run concurrently in PE row groups.
                        nc.tensor.matmul(
                            st[:, 0:TW],
                            lhsT=kt_sb(j)[0:64, :],
                            rhs=qt_sb[0:64, tsl],
                            start=True,
                            stop=True,
                        )
                        nc.tensor.matmul(
                            st[:, TW : 2 * TW],
                            lhsT=kt_sb(j)[64:128, :],
                            rhs=qt_sb[64:128, tsl],
                            start=True,
                            stop=True,
                        )
                        pt = ptpool.tile([P, 2 * TW], BF16, tag="pt")
                        if j in DVE_JS:
                            nc.vector.tensor_scalar(
                                pt[:].bitcast(I16),
                                st[:],
                                EXPA,
                                EXPB,
                                mybir.AluOpType.mult,
                                mybir.AluOpType.add,
                            )
                        elif j == DVE_HALF_J:
                            nc.scalar.activation(
                                pt[:, 0:TW], st[:, 0:TW],
                                mybir.ActivationFunctionType.Exp,
                            )
                            nc.vector.tensor_scalar(
                                pt[:, TW : 2 * TW].bitcast(I16),
                                st[:, TW : 2 * TW],
                                EXPA,
                                EXPB,
                                mybir.AluOpType.mult,
                                mybir.AluOpType.add,
                            )
                        else:
                            nc.scalar.activation(
                                pt[:], st[:], mybir.ActivationFunctionType.Exp
                            )
                        pts[j] = pt

                    def emit_pv(j):
                        pt = pts.pop(j)
                        nc.tensor.matmul(
                            ps_o[:],
                            lhsT=vp_sb(2 * j),
                            rhs=pt[:, 0:TW],
                            start=(j == 0),
                            stop=False,
                        )
                        nc.tensor.matmul(
                            ps_o[:],
                            lhsT=vp_sb(2 * j + 1),
                            rhs=pt[:, TW : 2 * TW],
                            start=False,
                            stop=(j == NS // 2 - 1),
                        )

                    for j in range(NS // 2):  # s-tile pairs (2j, 2j+1)
                        emit_qk_exp(j)
                        if j == LOOKAHEAD:
                            emit_finalize()
                        if j >= LOOKAHEAD:
                            emit_pv(j - LOOKAHEAD)
                    for j in range(NS // 2 - LOOKAHEAD, NS // 2):
                        emit_pv(j)
                    pending_fin[0] = (ps_o, hh, th)


            emit_finalize()

    nc.compile()
    return nc


def get_bass():
    if "nc" not in _BASS_CACHE:
        _BASS_CACHE["nc"] = _build_bass()
    return _BASS_CACHE["nc"]


def make_core_inputs(q, kv, core):
    """Host-side sharding + layout for one core: returns {inp}."""
    b = core // (N_CORES // B)
    h0 = HPC * (core % (N_CORES // B))
    inp = np.empty((HPC, P, INP_W), NP_BF16)
    for i in range(HPC):
        h = h0 + i
        Qt = np.ascontiguousarray(q[b, :, h, :].T)  # [64, 2048]
        inp[i, :64, 0:T] = Qt
        inp[i, 64:, 0:T] = Qt
        Kt = (kv[b, :, 0, h, :].astype(np.float32) * SCALE).T  # [64, 2048]
        Kts = Kt.reshape(64, NS, P)
        kt = inp[i, :, KT_OFF:VP_OFF].reshape(P, NS // 2, P)
        kt[:64] = Kts[:, 0::2]  # even s-tiles -> partitions 0-63
        kt[64:] = Kts[:, 1::2]  # odd s-tiles -> partitions 64-127
        V = kv[b, :, 1, h, :].reshape(NS, P, D)  # [s_tile, p, d]
        vp = inp[i, :, VP_OFF:].reshape(P, NS, D + 1)
        vp[:, :, :D] = V.transpose(1, 0, 2)
        vp[:, :, D] = 1.0
    return {"inp": inp}


def kernel(q, kv):
    global LAST_RESULT
    q = np.asarray(q, dtype=np.float32)
    kv = np.asarray(kv, dtype=np.float32)
    assert q.shape == (B, T, H, D) and kv.shape == (B, T, 2, H, D)

    nc = get_bass()
    in_maps = [make_core_inputs(q, kv, c) for c in range(N_CORES)]
    res = run_bass_kernel_spmd(nc, in_maps, core_ids=list(range(N_CORES)))
    LAST_RESULT = res

    out = np.empty((B, T, H, D), np.float32)
    for c in range(N_CORES):
        b = c // (N_CORES // B)
        h0 = HPC * (c % (N_CORES // B))
        out[b, :, h0 : h0 + HPC, :] = res.results[c]["out"]
    return out
